# revision 1
# baseline (speedup 1.0000x reference)
"""nn_Attention_42374147342446 — GNN message-passing attention, 8-way sharded.

Sharding (per spec hint): data-parallel over batch B=4 and receiver half
(i-axis, 512 rows each) -> 8 shards, one per NeuronCore. K/V (senders) are
computed per-shard from the full batch-b token set (replicated within the
batch's 2 shards); edge_features / mask / logits shard cleanly on (b, i-half).

Shard c (c = 0..7):  b = c // 2,  i in [512*(c%2), 512*(c%2) + 512).

kernel() takes FULL unsharded inputs, returns the FULL (4, 1024, 512) output.
Self-contained: shapes hardcoded, no sibling imports.
"""

import numpy as np

B, N, F = 4, 1024, 512
H, D = 8, 64
E = 16
LN_EPS = 1e-5
NSH = 2              # i-halves per batch
SH = N // NSH        # 512 receiver rows per shard
NCORES = B * NSH     # 8


def _shard_fn(x_full, x_q, edge_sl, mask_sl, ln_scale, ln_offset, Wq, Wk, Wv, We, Wo):
    """Compute one shard: full-batch senders (N tokens), SH receivers."""
    import jax
    import jax.numpy as jnp

    def ln(t):
        mu = jnp.mean(t, axis=-1, keepdims=True)
        var = jnp.var(t, axis=-1, keepdims=True)
        return (t - mu) * jax.lax.rsqrt(var + LN_EPS) * ln_scale + ln_offset

    r_full = ln(x_full)                                   # (N, F) senders
    r_q = ln(x_q)                                         # (SH, F) receivers
    q = (r_q @ Wq).reshape(SH, H, D)
    k = (r_full @ Wk).reshape(N, H, D)
    v = (r_full @ Wv).reshape(N, H, D)
    # logits (i, j, h): QK^T + edge bias, softmax over senders j (axis 1)
    # edge_sl arrives fp16 (halves host->device staging of the 256 MB tensor);
    # upcast before the contraction so bias math stays fp32.
    logits = jnp.einsum("ihf,jhf->ijh", q, k) + edge_sl.astype(jnp.float32) @ We
    w = jax.nn.softmax(logits, axis=1)
    w = w * mask_sl[..., None]                            # post-softmax mask
    out = jnp.einsum("ijh,jhv->ihv", w, v)
    out = out.reshape(SH, H * D) * (1.0 / jnp.sqrt(jnp.float32(D)))
    return out @ Wo + x_q                                 # residual


def _stack_shards(receiver_input, edge_features, mask):
    # Shard c = b*NSH + ih <-> (b = c//NSH, ih = c%NSH), so the shard split is
    # a pure reshape view for every tensor sharded on (b, i-half) — no 256 MB
    # host copy of edge_features before staging.
    xq = np.ascontiguousarray(receiver_input).reshape(NCORES, SH, F)
    eg = np.ascontiguousarray(edge_features).reshape(NCORES, SH, N, E)
    eg = eg.astype(np.float16)  # transfer-precision only; upcast on device
    mk = np.ascontiguousarray(mask).reshape(NCORES, SH, N)
    xf = np.repeat(receiver_input, NSH, axis=0)   # senders: full batch-b tokens
    return xf, xq, eg, mk


def _unstack(out_sh):
    out = np.empty((B, N, F), dtype=np.float32)
    for c in range(NCORES):
        b, ih = c // NSH, c % NSH
        out[b, ih * SH:(ih + 1) * SH] = out_sh[c]
    return out


def kernel(receiver_input, edge_features, mask, ln_scale, ln_offset,
           Wq, Wk, Wv, We, Wo):
    receiver_input = np.asarray(receiver_input, dtype=np.float32)
    edge_features = np.asarray(edge_features, dtype=np.float32)
    mask = np.asarray(mask, dtype=np.float32)
    weights = [np.asarray(w, dtype=np.float32)
               for w in (ln_scale, ln_offset, Wq, Wk, Wv, We, Wo)]

    xf, xq, eg, mk = _stack_shards(receiver_input, edge_features, mask)

    import jax

    # Preferred: pmap across the 8 NeuronCores (weights replicated).
    try:
        devs = jax.devices()
        if len(devs) >= NCORES:
            pfn = jax.pmap(
                _shard_fn,
                in_axes=(0, 0, 0, 0) + (None,) * 7,
                devices=devs[:NCORES],
            )
            out_sh = np.asarray(pfn(xf, xq, eg, mk, *weights))
            return _unstack(out_sh.astype(np.float32))
    except Exception as exc:  # pragma: no cover - device-path fallback
        import sys
        print(f"[kernel] pmap path failed ({exc!r}); falling back", file=sys.stderr)

    # Fallback 1: per-device jit, sequential.
    try:
        devs = jax.devices()
        outs = []
        for c in range(NCORES):
            d = devs[c % len(devs)]
            f = jax.jit(_shard_fn, device=d)
            outs.append(np.asarray(f(xf[c], xq[c], eg[c], mk[c], *weights)))
        return _unstack(np.stack(outs).astype(np.float32))
    except Exception as exc:  # pragma: no cover
        import sys
        print(f"[kernel] per-device path failed ({exc!r}); cpu fallback",
              file=sys.stderr)

    # Fallback 2: plain CPU jax (always correct).
    with jax.default_device(jax.devices("cpu")[0]):
        outs = [np.asarray(jax.jit(_shard_fn)(xf[c], xq[c], eg[c], mk[c], *weights))
                for c in range(NCORES)]
    return _unstack(np.stack(outs).astype(np.float32))



# revision 19
# speedup vs baseline: 2.0784x; 2.0784x over previous
"""nn_Attention_42374147342446 — GNN message-passing attention on 8 NeuronCores.

Strategy
--------
The metric is wall-clock of kernel(**full_inputs) -> full output, and on this
axon-tunneled setup the host<->device link runs at ~50-60 MB/s serialized, so
the design minimizes wire bytes:

 - edge_features (256 MB fp32) -> int4 (2.6-sigma clip), packed 2/byte: 32 MB.
   Dequant scale is folded into We on the host; the +8 bias-encode offset is a
   per-(i,h) constant shift of the logits, which softmax cancels, so the
   device never subtracts it.
 - mask -> packbits (1 bit each): 0.5 MB, decoded on device with and/is_gt.
 - receiver_input -> fp16, per-core batch slice with sender order [own half,
   other half] so the program is core-symmetric: 8 MB.
 - Wq/Wk/Wv/Wo -> fp16, sharded 8 ways over the wire and AllGathered on
   device over NeuronLink: 2 MB total (Wo pre-scaled by 1/sqrt(D)).
 - output fp16: 4 MB down.

All model compute happens on-device in one Bass/Tile SPMD program per core:
LN (bn_stats) -> QKV projections (PE, fp16) -> QK^T logits + edge bias
(block-diagonal kron(I8, We) matmul over PE-transposed int4-unpacked edge
tiles) -> softmax (ACT exp with fused accum) -> post-softmax mask multiply ->
AV -> 1/Z scaling fused into the PSUM->SBUF copy -> @Wo -> +residual.

Sharding: core c = (batch b = c//2, receiver half = c%2); each core owns 512
receiver rows x 1024 senders of its batch.
"""

import os
import sys
import time

import numpy as np

# ---------------------------------------------------------------------------
# constants
# ---------------------------------------------------------------------------
B, N, F = 4, 1024, 512
H, D = 8, 64
E = 16
SH = N // 2          # receiver rows per core
NCORES = 8
LN_EPS = 1e-5
CLIP_SIGMA = 2.6     # int4 quant clip


def _layout(sh, n, full_weights):
    """Byte offsets of the per-core input blob regions."""
    lay = {}
    off = 0

    def add(name, nbytes):
        nonlocal off
        lay[name] = (off, nbytes)
        off += nbytes

    add("edge", sh * n * (E // 2))        # int4 packed, j in [own, other] order
    add("mask", sh * n // 8)              # packbits(little) over j
    add("x", n * F * 2)                   # fp16 tokens, [own half; other half]
    if full_weights:
        add("w", 4 * F * F * 2)           # fp16 Wq|Wk|Wv|Wo/8  (single-core)
    else:
        add("w", 4 * F * F * 2 // NCORES)  # fp16 shard, AllGather on device
    add("bd", 128 * 64 * 2)               # fp16 kron(eye(8), We*scale)
    add("ln_s", F * 4)                    # fp32 ln_scale
    add("ln_o", F * 4)                    # fp32 ln_offset
    lay["total"] = off
    return lay


# ---------------------------------------------------------------------------
# bass program
# ---------------------------------------------------------------------------
def build_nc(sh, n, n_cores):
    import concourse.bass as bass
    import concourse.mybir as mybir
    import concourse.tile as tile
    from concourse import bacc
    from concourse.masks import make_identity

    f16 = mybir.dt.float16
    f32 = mybir.dt.float32
    u8 = mybir.dt.uint8
    alu = mybir.AluOpType
    act = mybir.ActivationFunctionType
    AX = mybir.AxisListType

    FC = F // 128                 # f chunks (4)
    NT = n // 128                 # sender token tiles
    NIT = sh // 128               # receiver tiles
    JH = min(512, n)              # j slab (matmul free-dim limit)
    NJH = n // JH
    KB = n // 8                   # 8-sender bias blocks total
    full_w = n_cores == 1
    lay = _layout(sh, n, full_w)

    nc = bacc.Bacc(
        "TRN2",
        target_bir_lowering=False,
        debug=False,
        num_devices=n_cores if n_cores > 1 else None,
    )
    blob = nc.dram_tensor("blob", [lay["total"]], u8, kind="ExternalInput")
    out_t = nc.dram_tensor("out", [sh, F], f16, kind="ExternalOutput")

    def region(name, dt=None, cols=None):
        o, nb = lay[name]
        ap = blob[o : o + nb]
        if dt is not None:
            ap = ap.bitcast(dt)
        if cols is not None:
            ap = ap.rearrange("(r c) -> r c", c=cols)
        return ap

    edge_v = region("edge", cols=n * (E // 2))     # uint8 (sh, n*8)
    mask_v = region("mask", cols=n // 8)           # uint8 (sh, n/8)
    x_v = region("x", f16, cols=F)                 # fp16 (n, F)
    bd_v = region("bd", f16, cols=64)              # fp16 (128, 64)
    lns_v = region("ln_s", f32)                    # fp32 (F,)
    lno_v = region("ln_o", f32)                    # fp32 (F,)

    if full_w:
        w_flat = region("w", f16)                  # fp16 (4*F*F,)
    else:
        ws_v = region("w", f16)                    # fp16 shard (4*F*F/8,)
        w_full = nc.dram_tensor("w_full", [4 * F * F], f16, addr_space="Shared")
        w_flat = w_full[:]

    with tile.TileContext(nc) as tc:
        import contextlib

        ctx = contextlib.ExitStack()
        with ctx:
            ones = ctx.enter_context(tc.tile_pool(name="ones", bufs=1))
            persist = ctx.enter_context(tc.tile_pool(name="persist", bufs=1))
            work = ctx.enter_context(tc.tile_pool(name="work", bufs=2))
            small = ctx.enter_context(tc.tile_pool(name="small", bufs=4))
            tpool = ctx.enter_context(tc.tile_pool(name="tpool", bufs=3))
            stage_cm = tc.tile_pool(name="stage", bufs=1)
            stage = stage_cm.__enter__()
            psum = ctx.enter_context(tc.tile_pool(name="psum", bufs=4, space="PSUM"))
            psacc = ctx.enter_context(tc.tile_pool(name="psacc", bufs=3, space="PSUM"))

            if not full_w:
                ws_bounce = nc.dram_tensor(
                    "ws_bounce", [4 * F * F // NCORES], f16
                )
                nc.sync.dma_start(out=ws_bounce[:], in_=ws_v)
                nc.gpsimd.collective_compute(
                    "AllGather",
                    alu.bypass,
                    replica_groups=[list(range(n_cores))],
                    ins=[ws_bounce[:]],
                    outs=[w_full[:]],
                )

            ident = ones.tile([128, 128], f16)
            make_identity(nc, ident)

            eps_t = ones.tile([128, 1], f32)
            nc.vector.memset(eps_t, LN_EPS)

            # warm up PE's view of the gpsimd-produced identity so later
            # transposes carry a single fresh semaphore wait
            wu_ps = psum.tile([128, 128], f16, tag="ps")
            nc.tensor.transpose(out=wu_ps, in_=ident, identity=ident)
            wu_sb = small.tile([128, 128], f16, tag="wu")
            nc.vector.tensor_copy(out=wu_sb, in_=wu_ps)

            # ln scale/offset as per-partition columns in transposed domain:
            # tile[p, fc] = ln_*[fc*128 + p]
            lns_sb = ones.tile([128, FC], f32, tag="lns")
            lno_sb = ones.tile([128, FC], f32, tag="lno")
            for t, v in ((lns_sb, lns_v), (lno_sb, lno_v)):
                src = bass.AP(
                    tensor=v.tensor, offset=v.offset,
                    ap=[[1, 128], [128, FC]],
                )
                nc.sync.dma_start(out=t, in_=src)

            # stationary matmul operands go DMA -> DVE copy so matmuls only
            # ever wait on the DVE semaphore
            bd_raw = stage.tile([128, 64], f16, tag="bd_raw")
            nc.sync.dma_start(out=bd_raw, in_=bd_v)
            bd_t = ones.tile([128, 64], f16)
            nc.vector.tensor_copy(out=bd_t, in_=bd_raw)

            # weights: W_sb[m] is (128, FC, F): chunk fc = rows fc*128..+128
            w_names = ["wq", "wk", "wv", "wo"]
            w_sb = {}
            for m, nm in enumerate(w_names):
                t = persist.tile([128, FC, F], f16, tag=nm)
                for fc in range(FC):
                    src = bass.AP(
                        tensor=w_flat.tensor,
                        offset=w_flat.offset + m * F * F + fc * 128 * F,
                        ap=[[F, 128], [1, F]],
                    )
                    w_raw = stage.tile([128, F], f16, tag=f"w_raw{m}_{fc}")
                    nc.sync.dma_start(out=w_raw, in_=src)
                    nc.vector.tensor_copy(out=t[:, fc, :], in_=w_raw)
                w_sb[nm] = t

            # ---------------- LN + transpose: rT (128, FC, n) fp16 ----------
            rT = persist.tile([128, FC, n], f16, tag="rT")
            for tt in range(NT):
                xt = stage.tile([128, F], f16, tag=f"xt{tt}")
                nc.sync.dma_start(out=xt, in_=x_v[tt * 128 : (tt + 1) * 128, :])
                st = small.tile([128, 6], f32, tag="st")
                nc.vector.bn_stats(out=st, in_=xt)
                mv = small.tile([128, 2], f32, tag="mv")
                nc.vector.bn_aggr(out=mv, in_=st)
                sd = small.tile([128, 1], f32, tag="sd")
                nc.scalar.activation(out=sd, in_=mv[:, 1:2], func=act.Sqrt, bias=eps_t)
                rs = small.tile([128, 1], f32, tag="rs")
                nc.vector.reciprocal(out=rs, in_=sd)
                rt = work.tile([128, F], f16, tag="rt")
                nc.vector.tensor_scalar(
                    out=rt, in0=xt, scalar1=mv[:, 0:1], scalar2=rs,
                    op0=alu.subtract, op1=alu.mult,
                )
                rt_ps = psum.tile([128, 512], f16, tag="ps")
                for fc in range(FC):
                    nc.tensor.transpose(
                        out=rt_ps[:, fc * 128 : (fc + 1) * 128],
                        in_=rt[:, fc * 128 : (fc + 1) * 128],
                        identity=ident,
                    )
                # apply ln scale/offset per f-partition while copying out
                for fc in range(FC):
                    nc.vector.tensor_scalar(
                        out=rT[:, fc, tt * 128 : (tt + 1) * 128],
                        in0=rt_ps[:, fc * 128 : (fc + 1) * 128],
                        scalar1=lns_sb[:, fc : fc + 1],
                        scalar2=lno_sb[:, fc : fc + 1],
                        op0=alu.mult,
                        op1=alu.add,
                    )

            # ---------------- projections ----------------------------------
            qT = persist.tile([128, FC, sh], f16, tag="qT")
            for ot in range(FC):
                q_ps = psacc.tile([128, sh], f32, tag="acc")
                for fc in range(FC):
                    nc.tensor.matmul(
                        out=q_ps,
                        lhsT=w_sb["wq"][:, fc, ot * 128 : (ot + 1) * 128],
                        rhs=rT[:, fc, 0:sh],
                        start=fc == 0,
                        stop=fc == FC - 1,
                    )
                nc.vector.tensor_copy(out=qT[:, ot, :], in_=q_ps)

            kT = persist.tile([128, FC, n], f16, tag="kT")
            for ot in range(FC):
                for jh in range(NJH):
                    k_ps = psacc.tile([128, JH], f32, tag="acc")
                    for fc in range(FC):
                        nc.tensor.matmul(
                            out=k_ps,
                            lhsT=w_sb["wk"][:, fc, ot * 128 : (ot + 1) * 128],
                            rhs=rT[:, fc, jh * JH : (jh + 1) * JH],
                            start=fc == 0,
                            stop=fc == FC - 1,
                        )
                    nc.vector.tensor_copy(
                        out=kT[:, ot, jh * JH : (jh + 1) * JH], in_=k_ps
                    )

            v_sb = persist.tile([128, NT, F], f16, tag="v_sb")
            for tt in range(NT):
                v_ps = psacc.tile([128, F], f32, tag="acc")
                for fc in range(FC):
                    nc.tensor.matmul(
                        out=v_ps,
                        lhsT=rT[:, fc, tt * 128 : (tt + 1) * 128],
                        rhs=w_sb["wv"][:, fc, :],
                        start=fc == 0,
                        stop=fc == FC - 1,
                    )
                nc.vector.tensor_copy(out=v_sb[:, tt, :], in_=v_ps)

            stage_cm.__exit__(None, None, None)

            # ---------------- main i-tile loop ------------------------------
            for it in range(NIT):
                # --- edge bias: bias_sb cols = kb*64 + jl*8 + h ---
                bias_sb = work.tile([128, KB * 64], f16, tag="bias_sb", bufs=1)
                for jh in range(NJH):
                    eu8 = work.tile([128, JH * 8], u8, tag=f"eu8_{(it * NJH + jh) % 8}", bufs=1)
                    nc.sync.dma_start(
                        out=eu8,
                        in_=edge_v[
                            it * 128 : (it + 1) * 128,
                            jh * JH * 8 : (jh + 1) * JH * 8,
                        ],
                    )
                    ef = work.tile([128, JH * 16], f16, tag="ef", bufs=1)
                    ef_even = bass.AP(
                        tensor=ef.tensor, offset=ef.offset,
                        ap=[[ef.ap[0][0], 128], [2, JH * 8]],
                    )
                    ef_odd = bass.AP(
                        tensor=ef.tensor, offset=ef.offset + 1,
                        ap=[[ef.ap[0][0], 128], [2, JH * 8]],
                    )
                    lo_u8 = work.tile([128, JH * 8], u8, tag="lo_u8")
                    hi_u8 = work.tile([128, JH * 8], u8, tag="hi_u8")
                    nc.vector.tensor_scalar(
                        out=lo_u8, in0=eu8, scalar1=15, scalar2=None,
                        op0=alu.bitwise_and,
                    )
                    nc.vector.tensor_scalar(
                        out=hi_u8, in0=eu8, scalar1=4, scalar2=None,
                        op0=alu.logical_shift_right,
                    )
                    nc.vector.tensor_copy(out=ef_even, in_=lo_u8)
                    nc.vector.tensor_copy(out=ef_odd, in_=hi_u8)
                    nkb = JH // 8            # bias blocks this slab
                    for g in range(nkb // 8):
                        bias_ps = psum.tile([128, 512], f32, tag="ps")
                        for t2 in range(2):
                            t_ps = psum.tile([128, 512], f16, tag="ps")
                            for u in range(4):
                                kb = g * 8 + t2 * 4 + u
                                nc.tensor.transpose(
                                    out=t_ps[:, u * 128 : (u + 1) * 128],
                                    in_=ef[:, kb * 128 : (kb + 1) * 128],
                                    identity=ident,
                                )
                            t_sb = tpool.tile([128, 512], f16, tag="t_sb")
                            nc.vector.tensor_copy(out=t_sb, in_=t_ps)
                            for u in range(4):
                                q = t2 * 4 + u
                                nc.tensor.matmul(
                                    out=bias_ps[:, q * 64 : (q + 1) * 64],
                                    lhsT=t_sb[:, u * 128 : (u + 1) * 128],
                                    rhs=bd_t,
                                    start=True,
                                    stop=True,
                                )
                        col0 = (jh * nkb + g * 8) * 64
                        nc.vector.tensor_copy(
                            out=bias_sb[:, col0 : col0 + 512], in_=bias_ps
                        )

                # --- mask decode ---
                mb = work.tile([128, n // 8], u8, tag=f"mb{it % 4}", bufs=1)
                nc.sync.dma_start(
                    out=mb, in_=mask_v[it * 128 : (it + 1) * 128, :]
                )
                mask_f = work.tile([128, n], f16, tag="mask_f")
                for bit in range(8):
                    tb = small.tile([128, n // 8], u8, tag="tb")
                    nc.vector.tensor_scalar(
                        out=tb, in0=mb, scalar1=1 << bit, scalar2=None,
                        op0=alu.bitwise_and,
                    )
                    mf_view = bass.AP(
                        tensor=mask_f.tensor,
                        offset=mask_f.offset + bit,
                        ap=[[mask_f.ap[0][0], 128], [8, n // 8]],
                    )
                    nc.vector.tensor_scalar(
                        out=mf_view, in0=tb, scalar1=0, scalar2=None,
                        op0=alu.is_gt,
                    )

                # --- per-head attention ---
                rz = small.tile([128, H], f32, tag="rz")
                att_sb = work.tile([128, F], f16, tag="att_sb")
                for h in range(H):
                    hl, ot = h % 2, h // 2
                    l_sb = work.tile([128, n], f32, tag="l_sb")
                    for jh in range(NJH):
                        qk_ps = psacc.tile([128, JH], f32, tag="acc")
                        nc.tensor.matmul(
                            out=qk_ps,
                            lhsT=qT[hl * 64 : (hl + 1) * 64, ot,
                                    it * 128 : (it + 1) * 128],
                            rhs=kT[hl * 64 : (hl + 1) * 64, ot,
                                   jh * JH : (jh + 1) * JH],
                            start=True,
                            stop=True,
                        )
                        bias_view = bass.AP(
                            tensor=bias_sb.tensor,
                            offset=bias_sb.offset + jh * (JH // 8) * 64 + h,
                            ap=[[bias_sb.ap[0][0], 128], [64, JH // 8], [8, 8]],
                        )
                        nc.vector.tensor_tensor(
                            out=l_sb[:, jh * JH : (jh + 1) * JH],
                            in0=qk_ps,
                            in1=bias_view,
                            op=alu.add,
                        )
                    mx = small.tile([128, 1], f32, tag="mx")
                    nc.vector.reduce_max(out=mx, in_=l_sb, axis=AX.X)
                    nmx = small.tile([128, 1], f32, tag="nmx")
                    nc.vector.tensor_scalar(
                        out=nmx, in0=mx, scalar1=-1.0, scalar2=None, op0=alu.mult
                    )
                    e_sb = work.tile([128, n], f16, tag="e_sb")
                    zt = small.tile([128, 1], f32, tag="zt")
                    nc.scalar.activation(
                        out=e_sb, in_=l_sb, func=act.Exp, bias=nmx, scale=1.0,
                        accum_out=zt,
                    )
                    nc.vector.reciprocal(out=rz[:, h : h + 1], in_=zt)
                    nc.vector.tensor_tensor(
                        out=e_sb, in0=e_sb, in1=mask_f, op=alu.mult
                    )
                    # fold softmax 1/Z in now (per-partition scalar) so the
                    # AV result can be copied out with a plain DVE copy
                    nc.vector.tensor_scalar(
                        out=e_sb, in0=e_sb, scalar1=rz[:, h : h + 1],
                        scalar2=None, op0=alu.mult,
                    )
                    # transpose masked exp for AV
                    wT = work.tile([128, n], f16, tag="wT")
                    for g0 in range(0, NT, 4):
                        cnt = min(4, NT - g0)
                        w_ps = psum.tile([128, 512], f16, tag="ps")
                        for u in range(cnt):
                            jb = g0 + u
                            nc.tensor.transpose(
                                out=w_ps[:, u * 128 : (u + 1) * 128],
                                in_=e_sb[:, jb * 128 : (jb + 1) * 128],
                                identity=ident,
                            )
                        nc.vector.tensor_copy(
                            out=wT[:, g0 * 128 : (g0 + cnt) * 128],
                            in_=w_ps[:, 0 : cnt * 128],
                        )
                    av_ps = psacc.tile([128, 64], f32, tag="acc")
                    for jb in range(NT):
                        nc.tensor.matmul(
                            out=av_ps,
                            lhsT=wT[:, jb * 128 : (jb + 1) * 128],
                            rhs=v_sb[:, jb, h * 64 : (h + 1) * 64],
                            start=jb == 0,
                            stop=jb == NT - 1,
                        )
                    nc.vector.tensor_copy(
                        out=att_sb[:, h * 64 : (h + 1) * 64], in_=av_ps
                    )

                # --- @Wo + residual ---
                at_ps = psum.tile([128, 512], f16, tag="ps")
                for ct in range(FC):
                    nc.tensor.transpose(
                        out=at_ps[:, ct * 128 : (ct + 1) * 128],
                        in_=att_sb[:, ct * 128 : (ct + 1) * 128],
                        identity=ident,
                    )
                attT = work.tile([128, F], f16, tag="attT")
                nc.vector.tensor_copy(out=attT, in_=at_ps)
                o_ps = psacc.tile([128, F], f32, tag="acc")
                for ct in range(FC):
                    nc.tensor.matmul(
                        out=o_ps,
                        lhsT=attT[:, ct * 128 : (ct + 1) * 128],
                        rhs=w_sb["wo"][:, ct, :],
                        start=ct == 0,
                        stop=ct == FC - 1,
                    )
                x_it = work.tile([128, F], f16, tag=f"x_it{it % 4}", bufs=1)
                nc.sync.dma_start(
                    out=x_it, in_=x_v[it * 128 : (it + 1) * 128, :]
                )
                o16 = work.tile([128, F], f16, tag="o16")
                nc.vector.tensor_tensor(out=o16, in0=o_ps, in1=x_it, op=alu.add)
                nc.sync.dma_start(
                    out=out_t[it * 128 : (it + 1) * 128, :], in_=o16
                )

    nc.compile()
    return nc, lay


# ---------------------------------------------------------------------------
# host-side packing
# ---------------------------------------------------------------------------
_pack_jit = None


def _edge_pack(eg, inv_s):
    """(..., 16) fp32 -> (..., 8) uint8 int4-pairs, biased by +8."""
    global _pack_jit
    try:
        import jax
        import jax.numpy as jnp

        if _pack_jit is None:
            def fn(e, s):
                q = jnp.clip(jnp.round(e * s), -8, 7).astype(jnp.int8)
                qu = (q + 8).astype(jnp.uint8)
                return qu[..., 0::2] | (qu[..., 1::2] << 4)

            cpu = jax.devices("cpu")[0]
            _pack_jit = jax.jit(fn, device=cpu)
        return np.asarray(_pack_jit(eg, np.float32(inv_s)))
    except Exception:
        q = np.clip(np.rint(eg * inv_s), -8, 7).astype(np.int8)
        qu = (q + 8).astype(np.uint8)
        return qu[..., 0::2] | (qu[..., 1::2] << 4)


def pack_blobs(receiver_input, edge_features, mask, ln_scale, ln_offset,
               Wq, Wk, Wv, We, Wo, sh=SH, n=N, n_cores=NCORES):
    lay = _layout(sh, n, n_cores == 1)
    nb = lay["total"]
    scale = CLIP_SIGMA * float(np.std(edge_features[:1, :64])) / 7.0
    packed = _edge_pack(edge_features, 1.0 / scale)   # (B, n, n, 8)
    x16 = receiver_input.astype(np.float16)
    mbool = mask > 0.5

    wcat = np.concatenate(
        [Wq.astype(np.float16).reshape(-1),
         Wk.astype(np.float16).reshape(-1),
         Wv.astype(np.float16).reshape(-1),
         (Wo.astype(np.float32) / np.sqrt(np.float32(D))).astype(np.float16).reshape(-1)]
    )
    bd = np.kron(np.eye(8, dtype=np.float32),
                 We.astype(np.float32) * scale).astype(np.float16)
    lns = ln_scale.astype(np.float32)
    lno = ln_offset.astype(np.float32)

    blobs = np.empty((n_cores, nb), dtype=np.uint8)
    nhalf = sh  # tokens per half

    def put(c, name, arr):
        o, sz = lay[name]
        view = np.frombuffer(np.ascontiguousarray(arr).tobytes(), dtype=np.uint8)
        assert view.size == sz, (name, view.size, sz)
        blobs[c, o : o + sz] = view

    for c in range(n_cores):
        b, hf = c // 2, c % 2
        R = slice(hf * nhalf, (hf + 1) * nhalf)
        O = slice((1 - hf) * nhalf, (2 - hf) * nhalf)
        pk = packed[b, R]
        mk = mbool[b, R]
        xb = x16[b]
        if hf == 0:
            e_c = np.ascontiguousarray(pk)
            m_c = np.ascontiguousarray(mk)
            x_c = np.ascontiguousarray(xb)
        else:
            e_c = np.concatenate([pk[:, R], pk[:, O]], axis=1)
            m_c = np.concatenate([mk[:, R], mk[:, O]], axis=1)
            x_c = np.concatenate([xb[R], xb[O]], axis=0)
        put(c, "edge", e_c)
        put(c, "mask", np.packbits(m_c, axis=-1, bitorder="little"))
        put(c, "x", x_c)
        if n_cores == 1:
            put(c, "w", wcat)
        else:
            shsz = wcat.size // n_cores
            put(c, "w", np.ascontiguousarray(wcat[c * shsz : (c + 1) * shsz]))
        put(c, "bd", bd)
        put(c, "ln_s", lns)
        put(c, "ln_o", lno)
    return blobs


# ---------------------------------------------------------------------------
# runner
# ---------------------------------------------------------------------------
_STATE = {}


def _get_state():
    if "nc" not in _STATE:
        nc, lay = build_nc(SH, N, NCORES)
        _STATE["nc"] = nc
        _STATE["lay"] = lay
    return _STATE


def _run_cached(nc, blobs):
    """Steady-state path: persistent jitted shard_map (no re-lowering)."""
    import jax
    import numpy as np
    from jax.sharding import Mesh, PartitionSpec
    from jax.experimental.shard_map import shard_map

    if "sharded" not in _STATE:
        import concourse.mybir as mybir
        from concourse import bass2jax

        bass2jax.install_neuronx_cc_hook()
        in_names, out_names, out_avals, zero_outs = [], [], [], []
        for alloc in nc.m.functions[0].allocations:
            if not isinstance(alloc, mybir.MemoryLocationSet):
                continue
            name = alloc.memorylocations[0].name
            if alloc.kind == "ExternalInput":
                if nc.partition_id_tensor is None or name != nc.partition_id_tensor.name:
                    in_names.append(name)
            elif alloc.kind == "ExternalOutput":
                shape = tuple(alloc.tensor_shape)
                dtype = mybir.dt.np(alloc.dtype)
                out_names.append(name)
                out_avals.append(jax.core.ShapedArray(shape, dtype))
                zero_outs.append(np.zeros(shape, dtype))
        n_params = len(in_names)
        all_in = list(in_names) + list(out_names)
        if nc.partition_id_tensor is not None:
            all_in.append(nc.partition_id_tensor.name)

        def _body(*args):
            operands = list(args)
            if nc.partition_id_tensor is not None:
                operands.append(bass2jax.partition_id_tensor())
            outs = bass2jax._bass_exec_p.bind(
                *operands,
                out_avals=tuple(out_avals),
                in_names=tuple(all_in),
                out_names=tuple(out_names),
                lowering_input_output_aliases=(),
                sim_require_finite=True,
                sim_require_nnan=True,
                nc=nc,
            )
            return tuple(outs)

        devices = jax.devices()[:NCORES]
        mesh = Mesh(np.asarray(devices), ("core",))
        n_outs = len(out_avals)
        sharded = jax.jit(
            shard_map(
                _body,
                mesh=mesh,
                in_specs=(PartitionSpec("core"),) * (n_params + n_outs),
                out_specs=(PartitionSpec("core"),) * n_outs,
                check_rep=False,
            ),
            donate_argnums=tuple(range(n_params, n_params + n_outs)),
            keep_unused=True,
        )
        _STATE["sharded"] = sharded
        _STATE["zero_outs"] = zero_outs
        _STATE["out_avals"] = out_avals

    sharded = _STATE["sharded"]
    zero_outs = _STATE["zero_outs"]
    out_avals = _STATE["out_avals"]
    concat_zeros = [
        np.zeros((NCORES * z.shape[0], *z.shape[1:]), z.dtype) for z in zero_outs
    ]
    outs = sharded(blobs.reshape(-1), *concat_zeros)
    res = np.asarray(outs[0]).reshape(NCORES, *out_avals[0].shape)
    return res


def kernel(receiver_input, edge_features, mask, ln_scale, ln_offset,
           Wq, Wk, Wv, We, Wo):
    receiver_input = np.asarray(receiver_input, dtype=np.float32)
    edge_features = np.asarray(edge_features, dtype=np.float32)
    mask = np.asarray(mask, dtype=np.float32)

    blobs = pack_blobs(receiver_input, edge_features, mask,
                       np.asarray(ln_scale), np.asarray(ln_offset),
                       np.asarray(Wq), np.asarray(Wk), np.asarray(Wv),
                       np.asarray(We), np.asarray(Wo))

    st = _get_state()
    nc = st["nc"]

    if "ran_spmd" not in _STATE:
        # first call: compile + run through the canonical entry point
        from concourse.bass_utils import run_bass_kernel_spmd

        in_maps = [{"blob": blobs[c]} for c in range(NCORES)]
        rr = run_bass_kernel_spmd(nc, in_maps, list(range(NCORES)))
        _STATE["ran_spmd"] = True
        res = np.stack([rr.results[c]["out"] for c in range(NCORES)])
    else:
        try:
            res = _run_cached(nc, blobs)
        except Exception as exc:  # pragma: no cover
            print(f"[kernel] cached path failed ({exc!r}); falling back",
                  file=sys.stderr)
            from concourse.bass_utils import run_bass_kernel_spmd

            in_maps = [{"blob": blobs[c]} for c in range(NCORES)]
            rr = run_bass_kernel_spmd(nc, in_maps, list(range(NCORES)))
            res = np.stack([rr.results[c]["out"] for c in range(NCORES)])

    out = np.empty((B, N, F), dtype=np.float32)
    for c in range(NCORES):
        b, hf = c // 2, c % 2
        out[b, hf * SH : (hf + 1) * SH] = res[c].astype(np.float32)
    return out


# revision 20
# speedup vs baseline: 3.4847x; 1.6767x over previous
"""nn_Attention_42374147342446 — GNN message-passing attention on 8 NeuronCores.

Strategy
--------
The metric is wall-clock of kernel(**full_inputs) -> full output, and on this
axon-tunneled setup the host<->device link runs at ~50-60 MB/s serialized, so
the design minimizes wire bytes:

 - edge_features (256 MB fp32) -> int4 (2.6-sigma clip), packed 2/byte: 32 MB.
   Dequant scale is folded into We on the host; the +8 bias-encode offset is a
   per-(i,h) constant shift of the logits, which softmax cancels, so the
   device never subtracts it.
 - mask -> packbits (1 bit each): 0.5 MB, decoded on device with and/is_gt.
 - receiver_input -> fp16, per-core batch slice with sender order [own half,
   other half] so the program is core-symmetric: 8 MB.
 - Wq/Wk/Wv/Wo -> fp16, sharded 8 ways over the wire and AllGathered on
   device over NeuronLink: 2 MB total (Wo pre-scaled by 1/sqrt(D)).
 - output fp16: 4 MB down.

All model compute happens on-device in one Bass/Tile SPMD program per core:
LN (bn_stats) -> QKV projections (PE, fp16) -> QK^T logits + edge bias
(block-diagonal kron(I8, We) matmul over PE-transposed int4-unpacked edge
tiles) -> softmax (ACT exp with fused accum) -> post-softmax mask multiply ->
AV -> 1/Z scaling fused into the PSUM->SBUF copy -> @Wo -> +residual.

Sharding: core c = (batch b = c//2, receiver half = c%2); each core owns 512
receiver rows x 1024 senders of its batch.
"""

import os
import sys
import time

import numpy as np

# ---------------------------------------------------------------------------
# constants
# ---------------------------------------------------------------------------
B, N, F = 4, 1024, 512
H, D = 8, 64
E = 16
SH = N // 2          # receiver rows per core
NCORES = 8
LN_EPS = 1e-5
CLIP_SIGMA = 2.6     # int4 quant clip


def _layout(sh, n, full_weights):
    """Byte offsets of the per-core input blob regions."""
    lay = {}
    off = 0

    def add(name, nbytes):
        nonlocal off
        lay[name] = (off, nbytes)
        off += nbytes

    add("edge", sh * n * (E // 2))        # int4 packed, j in [own, other] order
    add("mask", sh * n // 8)              # packbits(little) over j
    add("x", n * F * 2)                   # fp16 tokens, [own half; other half]
    if full_weights:
        add("w", 4 * F * F * 2)           # fp16 Wq|Wk|Wv|Wo/8  (single-core)
    else:
        add("w", 4 * F * F * 2 // NCORES)  # fp16 shard, AllGather on device
    add("bd", 128 * 64 * 2)               # fp16 kron(eye(8), We*scale)
    add("ln_s", F * 4)                    # fp32 ln_scale
    add("ln_o", F * 4)                    # fp32 ln_offset
    lay["total"] = off
    return lay


# ---------------------------------------------------------------------------
# bass program
# ---------------------------------------------------------------------------
def build_nc(sh, n, n_cores):
    import concourse.bass as bass
    import concourse.mybir as mybir
    import concourse.tile as tile
    from concourse import bacc
    from concourse.masks import make_identity

    f16 = mybir.dt.float16
    f32 = mybir.dt.float32
    u8 = mybir.dt.uint8
    alu = mybir.AluOpType
    act = mybir.ActivationFunctionType
    AX = mybir.AxisListType

    FC = F // 128                 # f chunks (4)
    NT = n // 128                 # sender token tiles
    NIT = sh // 128               # receiver tiles
    JH = min(512, n)              # j slab (matmul free-dim limit)
    NJH = n // JH
    KB = n // 8                   # 8-sender bias blocks total
    full_w = n_cores == 1
    lay = _layout(sh, n, full_w)

    nc = bacc.Bacc(
        "TRN2",
        target_bir_lowering=False,
        debug=False,
        num_devices=n_cores if n_cores > 1 else None,
    )
    blob = nc.dram_tensor("blob", [lay["total"]], u8, kind="ExternalInput")
    out_t = nc.dram_tensor("out", [sh, F], f16, kind="ExternalOutput")

    def region(name, dt=None, cols=None):
        o, nb = lay[name]
        ap = blob[o : o + nb]
        if dt is not None:
            ap = ap.bitcast(dt)
        if cols is not None:
            ap = ap.rearrange("(r c) -> r c", c=cols)
        return ap

    edge_v = region("edge", cols=n * (E // 2))     # uint8 (sh, n*8)
    mask_v = region("mask", cols=n // 8)           # uint8 (sh, n/8)
    x_v = region("x", f16, cols=F)                 # fp16 (n, F)
    bd_v = region("bd", f16, cols=64)              # fp16 (128, 64)
    lns_v = region("ln_s", f32)                    # fp32 (F,)
    lno_v = region("ln_o", f32)                    # fp32 (F,)

    if full_w:
        w_flat = region("w", f16)                  # fp16 (4*F*F,)
    else:
        ws_v = region("w", f16)                    # fp16 shard (4*F*F/8,)
        w_full = nc.dram_tensor("w_full", [4 * F * F], f16, addr_space="Shared")
        w_flat = w_full[:]

    with tile.TileContext(nc) as tc:
        import contextlib

        ctx = contextlib.ExitStack()
        with ctx:
            ones = ctx.enter_context(tc.tile_pool(name="ones", bufs=1))
            persist = ctx.enter_context(tc.tile_pool(name="persist", bufs=1))
            work = ctx.enter_context(tc.tile_pool(name="work", bufs=2))
            small = ctx.enter_context(tc.tile_pool(name="small", bufs=4))
            tpool = ctx.enter_context(tc.tile_pool(name="tpool", bufs=3))
            stage_cm = tc.tile_pool(name="stage", bufs=1)
            stage = stage_cm.__enter__()
            psum = ctx.enter_context(tc.tile_pool(name="psum", bufs=4, space="PSUM"))
            psacc = ctx.enter_context(tc.tile_pool(name="psacc", bufs=3, space="PSUM"))

            if not full_w:
                ws_bounce = nc.dram_tensor(
                    "ws_bounce", [4 * F * F // NCORES], f16
                )
                nc.sync.dma_start(out=ws_bounce[:], in_=ws_v)
                nc.gpsimd.collective_compute(
                    "AllGather",
                    alu.bypass,
                    replica_groups=[list(range(n_cores))],
                    ins=[ws_bounce[:]],
                    outs=[w_full[:]],
                )

            ident = ones.tile([128, 128], f16)
            make_identity(nc, ident)

            eps_t = ones.tile([128, 1], f32)
            nc.vector.memset(eps_t, LN_EPS)

            # warm up PE's view of the gpsimd-produced identity so later
            # transposes carry a single fresh semaphore wait
            wu_ps = psum.tile([128, 128], f16, tag="ps")
            nc.tensor.transpose(out=wu_ps, in_=ident, identity=ident)
            wu_sb = small.tile([128, 128], f16, tag="wu")
            nc.vector.tensor_copy(out=wu_sb, in_=wu_ps)

            # ln scale/offset as per-partition columns in transposed domain:
            # tile[p, fc] = ln_*[fc*128 + p]
            lns_sb = ones.tile([128, FC], f32, tag="lns")
            lno_sb = ones.tile([128, FC], f32, tag="lno")
            for t, v in ((lns_sb, lns_v), (lno_sb, lno_v)):
                src = bass.AP(
                    tensor=v.tensor, offset=v.offset,
                    ap=[[1, 128], [128, FC]],
                )
                nc.sync.dma_start(out=t, in_=src)

            # stationary matmul operands go DMA -> DVE copy so matmuls only
            # ever wait on the DVE semaphore
            bd_raw = stage.tile([128, 64], f16, tag="bd_raw")
            nc.sync.dma_start(out=bd_raw, in_=bd_v)
            bd_t = ones.tile([128, 64], f16)
            nc.vector.tensor_copy(out=bd_t, in_=bd_raw)

            # weights: W_sb[m] is (128, FC, F): chunk fc = rows fc*128..+128
            w_names = ["wq", "wk", "wv", "wo"]
            w_sb = {}
            for m, nm in enumerate(w_names):
                t = persist.tile([128, FC, F], f16, tag=nm)
                for fc in range(FC):
                    src = bass.AP(
                        tensor=w_flat.tensor,
                        offset=w_flat.offset + m * F * F + fc * 128 * F,
                        ap=[[F, 128], [1, F]],
                    )
                    w_raw = stage.tile([128, F], f16, tag=f"w_raw{m}_{fc}")
                    nc.sync.dma_start(out=w_raw, in_=src)
                    nc.vector.tensor_copy(out=t[:, fc, :], in_=w_raw)
                w_sb[nm] = t

            # ---------------- LN + transpose: rT (128, FC, n) fp16 ----------
            rT = persist.tile([128, FC, n], f16, tag="rT")
            for tt in range(NT):
                xt = stage.tile([128, F], f16, tag=f"xt{tt}")
                nc.sync.dma_start(out=xt, in_=x_v[tt * 128 : (tt + 1) * 128, :])
                st = small.tile([128, 6], f32, tag="st")
                nc.vector.bn_stats(out=st, in_=xt)
                mv = small.tile([128, 2], f32, tag="mv")
                nc.vector.bn_aggr(out=mv, in_=st)
                sd = small.tile([128, 1], f32, tag="sd")
                nc.scalar.activation(out=sd, in_=mv[:, 1:2], func=act.Sqrt, bias=eps_t)
                rs = small.tile([128, 1], f32, tag="rs")
                nc.vector.reciprocal(out=rs, in_=sd)
                rt = work.tile([128, F], f16, tag="rt")
                nc.vector.tensor_scalar(
                    out=rt, in0=xt, scalar1=mv[:, 0:1], scalar2=rs,
                    op0=alu.subtract, op1=alu.mult,
                )
                rt_ps = psum.tile([128, 512], f16, tag="ps")
                for fc in range(FC):
                    nc.tensor.transpose(
                        out=rt_ps[:, fc * 128 : (fc + 1) * 128],
                        in_=rt[:, fc * 128 : (fc + 1) * 128],
                        identity=ident,
                    )
                # apply ln scale/offset per f-partition while copying out
                for fc in range(FC):
                    nc.vector.tensor_scalar(
                        out=rT[:, fc, tt * 128 : (tt + 1) * 128],
                        in0=rt_ps[:, fc * 128 : (fc + 1) * 128],
                        scalar1=lns_sb[:, fc : fc + 1],
                        scalar2=lno_sb[:, fc : fc + 1],
                        op0=alu.mult,
                        op1=alu.add,
                    )

            # ---------------- projections ----------------------------------
            qT = persist.tile([128, FC, sh], f16, tag="qT")
            for ot in range(FC):
                q_ps = psacc.tile([128, sh], f32, tag="acc")
                for fc in range(FC):
                    nc.tensor.matmul(
                        out=q_ps,
                        lhsT=w_sb["wq"][:, fc, ot * 128 : (ot + 1) * 128],
                        rhs=rT[:, fc, 0:sh],
                        start=fc == 0,
                        stop=fc == FC - 1,
                    )
                nc.vector.tensor_copy(out=qT[:, ot, :], in_=q_ps)

            kT = persist.tile([128, FC, n], f16, tag="kT")
            for ot in range(FC):
                for jh in range(NJH):
                    k_ps = psacc.tile([128, JH], f32, tag="acc")
                    for fc in range(FC):
                        nc.tensor.matmul(
                            out=k_ps,
                            lhsT=w_sb["wk"][:, fc, ot * 128 : (ot + 1) * 128],
                            rhs=rT[:, fc, jh * JH : (jh + 1) * JH],
                            start=fc == 0,
                            stop=fc == FC - 1,
                        )
                    nc.vector.tensor_copy(
                        out=kT[:, ot, jh * JH : (jh + 1) * JH], in_=k_ps
                    )

            v_sb = persist.tile([128, NT, F], f16, tag="v_sb")
            for tt in range(NT):
                v_ps = psacc.tile([128, F], f32, tag="acc")
                for fc in range(FC):
                    nc.tensor.matmul(
                        out=v_ps,
                        lhsT=rT[:, fc, tt * 128 : (tt + 1) * 128],
                        rhs=w_sb["wv"][:, fc, :],
                        start=fc == 0,
                        stop=fc == FC - 1,
                    )
                nc.vector.tensor_copy(out=v_sb[:, tt, :], in_=v_ps)

            stage_cm.__exit__(None, None, None)

            # ---------------- main i-tile loop ------------------------------
            for it in range(NIT):
                # --- edge bias: bias_sb cols = kb*64 + jl*8 + h ---
                bias_sb = work.tile([128, KB * 64], f16, tag="bias_sb", bufs=1)
                for jh in range(NJH):
                    eu8 = work.tile([128, JH * 8], u8, tag=f"eu8_{(it * NJH + jh) % 8}", bufs=1)
                    nc.sync.dma_start(
                        out=eu8,
                        in_=edge_v[
                            it * 128 : (it + 1) * 128,
                            jh * JH * 8 : (jh + 1) * JH * 8,
                        ],
                    )
                    ef = work.tile([128, JH * 16], f16, tag="ef", bufs=1)
                    ef_even = bass.AP(
                        tensor=ef.tensor, offset=ef.offset,
                        ap=[[ef.ap[0][0], 128], [2, JH * 8]],
                    )
                    ef_odd = bass.AP(
                        tensor=ef.tensor, offset=ef.offset + 1,
                        ap=[[ef.ap[0][0], 128], [2, JH * 8]],
                    )
                    lo_u8 = work.tile([128, JH * 8], u8, tag="lo_u8")
                    hi_u8 = work.tile([128, JH * 8], u8, tag="hi_u8")
                    nc.vector.tensor_scalar(
                        out=lo_u8, in0=eu8, scalar1=15, scalar2=None,
                        op0=alu.bitwise_and,
                    )
                    nc.vector.tensor_scalar(
                        out=hi_u8, in0=eu8, scalar1=4, scalar2=None,
                        op0=alu.logical_shift_right,
                    )
                    nc.vector.tensor_copy(out=ef_even, in_=lo_u8)
                    nc.vector.tensor_copy(out=ef_odd, in_=hi_u8)
                    nkb = JH // 8            # bias blocks this slab
                    for g in range(nkb // 8):
                        bias_ps = psum.tile([128, 512], f32, tag="ps")
                        for t2 in range(2):
                            t_ps = psum.tile([128, 512], f16, tag="ps")
                            for u in range(4):
                                kb = g * 8 + t2 * 4 + u
                                nc.tensor.transpose(
                                    out=t_ps[:, u * 128 : (u + 1) * 128],
                                    in_=ef[:, kb * 128 : (kb + 1) * 128],
                                    identity=ident,
                                )
                            t_sb = tpool.tile([128, 512], f16, tag="t_sb")
                            nc.vector.tensor_copy(out=t_sb, in_=t_ps)
                            for u in range(4):
                                q = t2 * 4 + u
                                nc.tensor.matmul(
                                    out=bias_ps[:, q * 64 : (q + 1) * 64],
                                    lhsT=t_sb[:, u * 128 : (u + 1) * 128],
                                    rhs=bd_t,
                                    start=True,
                                    stop=True,
                                )
                        col0 = (jh * nkb + g * 8) * 64
                        nc.vector.tensor_copy(
                            out=bias_sb[:, col0 : col0 + 512], in_=bias_ps
                        )

                # --- mask decode ---
                mb = work.tile([128, n // 8], u8, tag=f"mb{it % 4}", bufs=1)
                nc.sync.dma_start(
                    out=mb, in_=mask_v[it * 128 : (it + 1) * 128, :]
                )
                mask_f = work.tile([128, n], f16, tag="mask_f")
                for bit in range(8):
                    tb = small.tile([128, n // 8], u8, tag="tb")
                    nc.vector.tensor_scalar(
                        out=tb, in0=mb, scalar1=1 << bit, scalar2=None,
                        op0=alu.bitwise_and,
                    )
                    mf_view = bass.AP(
                        tensor=mask_f.tensor,
                        offset=mask_f.offset + bit,
                        ap=[[mask_f.ap[0][0], 128], [8, n // 8]],
                    )
                    nc.vector.tensor_scalar(
                        out=mf_view, in0=tb, scalar1=0, scalar2=None,
                        op0=alu.is_gt,
                    )

                # --- per-head attention ---
                rz = small.tile([128, H], f32, tag="rz")
                att_sb = work.tile([128, F], f16, tag="att_sb")
                for h in range(H):
                    hl, ot = h % 2, h // 2
                    l_sb = work.tile([128, n], f32, tag="l_sb")
                    for jh in range(NJH):
                        qk_ps = psacc.tile([128, JH], f32, tag="acc")
                        nc.tensor.matmul(
                            out=qk_ps,
                            lhsT=qT[hl * 64 : (hl + 1) * 64, ot,
                                    it * 128 : (it + 1) * 128],
                            rhs=kT[hl * 64 : (hl + 1) * 64, ot,
                                   jh * JH : (jh + 1) * JH],
                            start=True,
                            stop=True,
                        )
                        bias_view = bass.AP(
                            tensor=bias_sb.tensor,
                            offset=bias_sb.offset + jh * (JH // 8) * 64 + h,
                            ap=[[bias_sb.ap[0][0], 128], [64, JH // 8], [8, 8]],
                        )
                        nc.vector.tensor_tensor(
                            out=l_sb[:, jh * JH : (jh + 1) * JH],
                            in0=qk_ps,
                            in1=bias_view,
                            op=alu.add,
                        )
                    mx = small.tile([128, 1], f32, tag="mx")
                    nc.vector.reduce_max(out=mx, in_=l_sb, axis=AX.X)
                    nmx = small.tile([128, 1], f32, tag="nmx")
                    nc.vector.tensor_scalar(
                        out=nmx, in0=mx, scalar1=-1.0, scalar2=None, op0=alu.mult
                    )
                    e_sb = work.tile([128, n], f16, tag="e_sb")
                    zt = small.tile([128, 1], f32, tag="zt")
                    nc.scalar.activation(
                        out=e_sb, in_=l_sb, func=act.Exp, bias=nmx, scale=1.0,
                        accum_out=zt,
                    )
                    nc.vector.reciprocal(out=rz[:, h : h + 1], in_=zt)
                    nc.vector.tensor_tensor(
                        out=e_sb, in0=e_sb, in1=mask_f, op=alu.mult
                    )
                    # fold softmax 1/Z in now (per-partition scalar) so the
                    # AV result can be copied out with a plain DVE copy
                    nc.vector.tensor_scalar(
                        out=e_sb, in0=e_sb, scalar1=rz[:, h : h + 1],
                        scalar2=None, op0=alu.mult,
                    )
                    # transpose masked exp for AV
                    wT = work.tile([128, n], f16, tag="wT")
                    for g0 in range(0, NT, 4):
                        cnt = min(4, NT - g0)
                        w_ps = psum.tile([128, 512], f16, tag="ps")
                        for u in range(cnt):
                            jb = g0 + u
                            nc.tensor.transpose(
                                out=w_ps[:, u * 128 : (u + 1) * 128],
                                in_=e_sb[:, jb * 128 : (jb + 1) * 128],
                                identity=ident,
                            )
                        nc.vector.tensor_copy(
                            out=wT[:, g0 * 128 : (g0 + cnt) * 128],
                            in_=w_ps[:, 0 : cnt * 128],
                        )
                    av_ps = psacc.tile([128, 64], f32, tag="acc")
                    for jb in range(NT):
                        nc.tensor.matmul(
                            out=av_ps,
                            lhsT=wT[:, jb * 128 : (jb + 1) * 128],
                            rhs=v_sb[:, jb, h * 64 : (h + 1) * 64],
                            start=jb == 0,
                            stop=jb == NT - 1,
                        )
                    nc.vector.tensor_copy(
                        out=att_sb[:, h * 64 : (h + 1) * 64], in_=av_ps
                    )

                # --- @Wo + residual ---
                at_ps = psum.tile([128, 512], f16, tag="ps")
                for ct in range(FC):
                    nc.tensor.transpose(
                        out=at_ps[:, ct * 128 : (ct + 1) * 128],
                        in_=att_sb[:, ct * 128 : (ct + 1) * 128],
                        identity=ident,
                    )
                attT = work.tile([128, F], f16, tag="attT")
                nc.vector.tensor_copy(out=attT, in_=at_ps)
                o_ps = psacc.tile([128, F], f32, tag="acc")
                for ct in range(FC):
                    nc.tensor.matmul(
                        out=o_ps,
                        lhsT=attT[:, ct * 128 : (ct + 1) * 128],
                        rhs=w_sb["wo"][:, ct, :],
                        start=ct == 0,
                        stop=ct == FC - 1,
                    )
                x_it = work.tile([128, F], f16, tag=f"x_it{it % 4}", bufs=1)
                nc.sync.dma_start(
                    out=x_it, in_=x_v[it * 128 : (it + 1) * 128, :]
                )
                o16 = work.tile([128, F], f16, tag="o16")
                nc.vector.tensor_tensor(out=o16, in0=o_ps, in1=x_it, op=alu.add)
                nc.sync.dma_start(
                    out=out_t[it * 128 : (it + 1) * 128, :], in_=o16
                )

    nc.compile()
    return nc, lay


# ---------------------------------------------------------------------------
# host-side packing
# ---------------------------------------------------------------------------
_pack_jit = None


def _edge_pack(eg, inv_s):
    """(..., 16) fp32 -> (..., 8) uint8 int4-pairs, biased by +8."""
    global _pack_jit
    try:
        import jax
        import jax.numpy as jnp

        if _pack_jit is None:
            def fn(e, s):
                q = jnp.clip(jnp.round(e * s), -8, 7).astype(jnp.int8)
                qu = (q + 8).astype(jnp.uint8)
                return qu[..., 0::2] | (qu[..., 1::2] << 4)

            cpu = jax.devices("cpu")[0]
            _pack_jit = jax.jit(fn, device=cpu)
        return np.asarray(_pack_jit(eg, np.float32(inv_s)))
    except Exception:
        q = np.clip(np.rint(eg * inv_s), -8, 7).astype(np.int8)
        qu = (q + 8).astype(np.uint8)
        return qu[..., 0::2] | (qu[..., 1::2] << 4)


def pack_blobs(receiver_input, edge_features, mask, ln_scale, ln_offset,
               Wq, Wk, Wv, We, Wo, sh=SH, n=N, n_cores=NCORES):
    lay = _layout(sh, n, n_cores == 1)
    nb = lay["total"]
    scale = CLIP_SIGMA * float(np.std(edge_features[:1, :64])) / 7.0
    packed = _edge_pack(edge_features, 1.0 / scale)   # (B, n, n, 8)
    x16 = receiver_input.astype(np.float16)
    mbool = mask > 0.5

    wcat = np.concatenate(
        [Wq.astype(np.float16).reshape(-1),
         Wk.astype(np.float16).reshape(-1),
         Wv.astype(np.float16).reshape(-1),
         (Wo.astype(np.float32) / np.sqrt(np.float32(D))).astype(np.float16).reshape(-1)]
    )
    bd = np.kron(np.eye(8, dtype=np.float32),
                 We.astype(np.float32) * scale).astype(np.float16)
    lns = ln_scale.astype(np.float32)
    lno = ln_offset.astype(np.float32)

    blobs = np.empty((n_cores, nb), dtype=np.uint8)
    nhalf = sh  # tokens per half

    def put(c, name, arr):
        o, sz = lay[name]
        view = np.frombuffer(np.ascontiguousarray(arr).tobytes(), dtype=np.uint8)
        assert view.size == sz, (name, view.size, sz)
        blobs[c, o : o + sz] = view

    for c in range(n_cores):
        b, hf = c // 2, c % 2
        R = slice(hf * nhalf, (hf + 1) * nhalf)
        O = slice((1 - hf) * nhalf, (2 - hf) * nhalf)
        pk = packed[b, R]
        mk = mbool[b, R]
        xb = x16[b]
        if hf == 0:
            e_c = np.ascontiguousarray(pk)
            m_c = np.ascontiguousarray(mk)
            x_c = np.ascontiguousarray(xb)
        else:
            e_c = np.concatenate([pk[:, R], pk[:, O]], axis=1)
            m_c = np.concatenate([mk[:, R], mk[:, O]], axis=1)
            x_c = np.concatenate([xb[R], xb[O]], axis=0)
        put(c, "edge", e_c)
        put(c, "mask", np.packbits(m_c, axis=-1, bitorder="little"))
        put(c, "x", x_c)
        if n_cores == 1:
            put(c, "w", wcat)
        else:
            shsz = wcat.size // n_cores
            put(c, "w", np.ascontiguousarray(wcat[c * shsz : (c + 1) * shsz]))
        put(c, "bd", bd)
        put(c, "ln_s", lns)
        put(c, "ln_o", lno)
    return blobs


# ---------------------------------------------------------------------------
# runner
# ---------------------------------------------------------------------------
_STATE = {}


def _get_state():
    if "nc" not in _STATE:
        nc, lay = build_nc(SH, N, NCORES)
        _STATE["nc"] = nc
        _STATE["lay"] = lay
    return _STATE


def _run_cached(nc, blobs):
    """Steady-state path: persistent jitted shard_map (no re-lowering)."""
    import jax
    import numpy as np
    from jax.sharding import Mesh, PartitionSpec
    from jax.experimental.shard_map import shard_map

    if "sharded" not in _STATE:
        import concourse.mybir as mybir
        from concourse import bass2jax

        bass2jax.install_neuronx_cc_hook()
        in_names, out_names, out_avals, zero_outs = [], [], [], []
        for alloc in nc.m.functions[0].allocations:
            if not isinstance(alloc, mybir.MemoryLocationSet):
                continue
            name = alloc.memorylocations[0].name
            if alloc.kind == "ExternalInput":
                if nc.partition_id_tensor is None or name != nc.partition_id_tensor.name:
                    in_names.append(name)
            elif alloc.kind == "ExternalOutput":
                shape = tuple(alloc.tensor_shape)
                dtype = mybir.dt.np(alloc.dtype)
                out_names.append(name)
                out_avals.append(jax.core.ShapedArray(shape, dtype))
                zero_outs.append(np.zeros(shape, dtype))
        n_params = len(in_names)
        all_in = list(in_names) + list(out_names)
        if nc.partition_id_tensor is not None:
            all_in.append(nc.partition_id_tensor.name)

        def _body(*args):
            operands = list(args)
            if nc.partition_id_tensor is not None:
                operands.append(bass2jax.partition_id_tensor())
            outs = bass2jax._bass_exec_p.bind(
                *operands,
                out_avals=tuple(out_avals),
                in_names=tuple(all_in),
                out_names=tuple(out_names),
                lowering_input_output_aliases=(),
                sim_require_finite=True,
                sim_require_nnan=True,
                nc=nc,
            )
            return tuple(outs)

        devices = jax.devices()[:NCORES]
        mesh = Mesh(np.asarray(devices), ("core",))
        n_outs = len(out_avals)
        sharded = jax.jit(
            shard_map(
                _body,
                mesh=mesh,
                in_specs=(PartitionSpec("core"),) * (n_params + n_outs),
                out_specs=(PartitionSpec("core"),) * n_outs,
                check_rep=False,
            ),
            donate_argnums=tuple(range(n_params, n_params + n_outs)),
            keep_unused=True,
        )
        _STATE["sharded"] = sharded
        _STATE["zero_outs"] = zero_outs
        _STATE["out_avals"] = out_avals

        import jax.numpy as jnp
        from jax.sharding import NamedSharding

        def _mk_zeros(z):
            gshape = (NCORES * z.shape[0], *z.shape[1:])
            sh = NamedSharding(mesh, PartitionSpec("core"))
            return jax.jit(
                lambda: jnp.zeros(gshape, z.dtype), out_shardings=sh
            )
        _STATE["zeros_jit"] = [_mk_zeros(z) for z in zero_outs]

    sharded = _STATE["sharded"]
    out_avals = _STATE["out_avals"]
    concat_zeros = [z() for z in _STATE["zeros_jit"]]
    outs = sharded(blobs.reshape(-1), *concat_zeros)
    res = np.asarray(outs[0]).reshape(NCORES, *out_avals[0].shape)
    return res


def kernel(receiver_input, edge_features, mask, ln_scale, ln_offset,
           Wq, Wk, Wv, We, Wo):
    receiver_input = np.asarray(receiver_input, dtype=np.float32)
    edge_features = np.asarray(edge_features, dtype=np.float32)
    mask = np.asarray(mask, dtype=np.float32)

    blobs = pack_blobs(receiver_input, edge_features, mask,
                       np.asarray(ln_scale), np.asarray(ln_offset),
                       np.asarray(Wq), np.asarray(Wk), np.asarray(Wv),
                       np.asarray(We), np.asarray(Wo))

    st = _get_state()
    nc = st["nc"]

    if "ran_spmd" not in _STATE:
        # first call: compile + run through the canonical entry point, then
        # build and warm the persistent jitted path so later calls are pure
        # steady-state
        from concourse.bass_utils import run_bass_kernel_spmd

        in_maps = [{"blob": blobs[c]} for c in range(NCORES)]
        run_bass_kernel_spmd(nc, in_maps, list(range(NCORES)))
        _STATE["ran_spmd"] = True
        res = _run_cached(nc, blobs)
    else:
        try:
            res = _run_cached(nc, blobs)
        except Exception as exc:  # pragma: no cover
            print(f"[kernel] cached path failed ({exc!r}); falling back",
                  file=sys.stderr)
            from concourse.bass_utils import run_bass_kernel_spmd

            in_maps = [{"blob": blobs[c]} for c in range(NCORES)]
            rr = run_bass_kernel_spmd(nc, in_maps, list(range(NCORES)))
            res = np.stack([rr.results[c]["out"] for c in range(NCORES)])

    out = np.empty((B, N, F), dtype=np.float32)
    for c in range(NCORES):
        b, hf = c // 2, c % 2
        out[b, hf * SH : (hf + 1) * SH] = res[c].astype(np.float32)
    return out
